# revision 16
# baseline (speedup 1.0000x reference)
"""GATv2 self-attention kernel for 8 Trainium2 NeuronCores.

Sharding: one attention head per core (8 heads / 8 cores). Each core computes
its head's attn-weighted projection as a partial sum over heads, the cores
ReduceScatter the partials over the feature axis, and each core finishes its
256-column feature slice (bias-mean + residual) and returns it; the host
concatenates the 8 slices.

Math per head h (reference):
  X = inputs.reshape(B*S, F); x0 = rows of X with s == 0
  Wh = leaky_relu(X @ W2h + broadcast_s(x0 @ W1h))      [B*S, F]
  e  = Wh @ att_w[h]; attn = softmax_s(e)
  out = sum_h (attn * Wh)/H + mean_h(bias) + X

All matmuls run in float32r (TF32-like, 11-bit mantissa) on the PE array.
X is transposed on-chip via PE transpose-mode matmuls. The broadcast x0@W1
term is accumulated into the same PSUM group as the X@W2 tiles through a
0/1 selector matmul, so no elementwise broadcast-add pass is needed.
"""

import sys
import os
import numpy as np

sys.path.insert(0, "/opt/trn_rl_repo")

B, S, F, H = 256, 8, 2048, 8
BS = B * S            # 2048
NB = 1024             # bs-chunk size (2 chunks)
NCHUNK = BS // NB     # 2
FSLICE = F // H       # 256 output feature columns per core
ALPHA = 0.3

_cache = {}


def _build(reps=1):
    import concourse.bacc as bacc
    import concourse.mybir as mybir
    import concourse.tile as tile
    import concourse.bass as bass
    from concourse.masks import make_identity

    f32 = mybir.dt.float32
    f32r = mybir.dt.float32r
    AF = mybir.ActivationFunctionType
    OP = mybir.AluOpType

    nc = bacc.Bacc(num_devices=H)

    w1_in = nc.declare_dram_parameter("w1", [F, F], f32, isOutput=False)
    w2_in = nc.declare_dram_parameter("w2", [F, F], f32, isOutput=False)
    x_in = nc.declare_dram_parameter("x", [BS, F], f32, isOutput=False)
    attw_in = nc.declare_dram_parameter("attw", [F], f32, isOutput=False)
    sel_in = nc.declare_dram_parameter("sel", [128, NB], f32, isOutput=False)
    xres_in = nc.declare_dram_parameter("xres", [BS, FSLICE], f32, isOutput=False)
    bm_in = nc.declare_dram_parameter("bm", [FSLICE], f32, isOutput=False)
    out_ext = nc.declare_dram_parameter("out", [BS, FSLICE], f32, isOutput=True)

    from contextlib import ExitStack
    with tile.TileContext(nc) as tc:
        with ExitStack() as ctx:
            consts = ctx.enter_context(tc.tile_pool(name="consts", bufs=1))
            xtp = ctx.enter_context(tc.tile_pool(name="xtp", bufs=1))
            whp = ctx.enter_context(tc.tile_pool(name="whp", bufs=1))
            wblkp = ctx.enter_context(tc.tile_pool(name="wblk", bufs=1))
            wrp = ctx.enter_context(tc.tile_pool(name="wrp", bufs=2))
            x0np = ctx.enter_context(tc.tile_pool(name="x0n", bufs=1))
            xnatp = ctx.enter_context(tc.tile_pool(name="xnat", bufs=2))
            t03p = ctx.enter_context(tc.tile_pool(name="t03", bufs=2))
            stagep = ctx.enter_context(tc.tile_pool(name="stage", bufs=2))
            esmp = ctx.enter_context(tc.tile_pool(name="esm", bufs=1))
            abrsbp = ctx.enter_context(tc.tile_pool(name="abrsb", bufs=1))
            xrsp = ctx.enter_context(tc.tile_pool(name="xrs", bufs=2))
            outstp = ctx.enter_context(tc.tile_pool(name="outst", bufs=1))
            ypool = ctx.enter_context(tc.tile_pool(name="ypool", bufs=4, space="PSUM"))
            epool = ctx.enter_context(tc.tile_pool(name="epool", bufs=2, space="PSUM"))
            tpool = ctx.enter_context(tc.tile_pool(name="tpool", bufs=2, space="PSUM"))
            dpool = ctx.enter_context(tc.tile_pool(name="dram", bufs=2, space="DRAM"))

            # ---------------- constants ----------------
            ident = consts.tile([128, 128], f32)
            make_identity(nc, ident)

            attw_f = consts.tile([128, F // 128], f32)
            nc.sync.dma_start(out=attw_f, in_=attw_in.rearrange("(o p) -> p o", p=128))
            attw_r = consts.tile([128, F // 128], f32r)
            nc.vector.tensor_copy(attw_r, attw_f)

            sel_r = consts.tile([128, NB], f32r)
            for half in range(NB // 512):
                sh = stagep.tile([128, 512], f32, tag="st")
                nc.sync.dma_start(out=sh, in_=sel_in[:, half * 512:(half + 1) * 512])
                nc.vector.tensor_copy(sel_r[:, half * 512:(half + 1) * 512], sh)

            bm_sb = consts.tile([128, FSLICE // 128], f32)
            nc.sync.dma_start(out=bm_sb, in_=bm_in.rearrange("(o p) -> p o", p=128))

            # view of x grouped by (b, s)
            x_bsf = x_in.rearrange("(b s) f -> b s f", s=S)

            for _rep in range(reps):
                _run_body(nc, tc, mybir, bass, f32, f32r, AF, OP,
                          make_identity, ident, attw_r, sel_r, bm_sb, x_bsf,
                          w1_in, w2_in, x_in, xres_in, out_ext,
                          consts, xtp, whp, wblkp, wrp, x0np, xnatp, t03p,
                          stagep, esmp, abrsbp, xrsp, outstp,
                          ypool, epool, tpool, dpool, _rep)

    nc.compile()
    return nc


def _run_body(nc, tc, mybir, bass, f32, f32r, AF, OP,
              make_identity, ident, attw_r, sel_r, bm_sb, x_bsf,
              w1_in, w2_in, x_in, xres_in, out_ext,
              consts, xtp, whp, wblkp, wrp, x0np, xnatp, t03p,
              stagep, esmp, abrsbp, xrsp, outstp,
              ypool, epool, tpool, dpool, rep):
    if True:
        if True:
            # ---------------- prologue: X0 = x0 @ W1 (natural [b, f] layout) --------
            # x0t: [128 fi, 16 fi_outer, 256 b] carved out of the Wh buffer slot
            x0t_full = whp.tile([128, F // 128, NB], f32r, tag="wh")
            x0t = x0t_full[:, :, :B]
            for bt in range(B // 128):
                for hf in range(2):
                    x0nat = xnatp.tile([128, F // 2], f32, tag="xnat")
                    nc.sync.dma_start(
                        out=x0nat,
                        in_=x_bsf[bt * 128:(bt + 1) * 128, 0,
                                  hf * (F // 2):(hf + 1) * (F // 2)])
                    for fj in range(F // 256):
                        fi = hf * (F // 256) + fj
                        pt = tpool.tile([128, 128], f32, tag="tp")
                        nc.tensor.transpose(
                            pt, x0nat[:, fj * 128:(fj + 1) * 128], ident)
                        nc.any.tensor_copy(
                            out=x0t[:, fi, bt * 128:(bt + 1) * 128], in_=pt)

            # X0 in transposed layout first: X0_T [128 fo, 16 fo_outer, 256 b],
            # stored in spare columns of the same wh-tag slot as x0t.
            x0T = x0t_full[:, :, B:2 * B]
            WCOLS = 128  # fo columns per W-block load
            for fb in range(F // WCOLS):
                wblk = wblkp.tile([128, F // 128, WCOLS], f32, tag="wblk")
                nc.sync.dma_start(
                    out=wblk,
                    in_=w1_in.rearrange("(ko kp) n -> kp ko n", kp=128)[
                        :, :, fb * WCOLS:(fb + 1) * WCOLS],
                )
                wr = wrp.tile([128, F // 128, WCOLS], f32r, tag="wr")
                nc.vector.tensor_copy(wr, wblk)
                ps_full = ypool.tile([128, 512], f32, tag="yp")
                ps = ps_full[:, :B]
                for fi in range(F // 128):
                    nc.tensor.matmul(
                        ps,
                        wr[:, fi, :],
                        x0t[:, fi, :],
                        start=(fi == 0),
                        stop=(fi == F // 128 - 1),
                    )
                nc.any.tensor_copy(out=x0T[:, fb, :], in_=ps)

            # transpose X0_T back to natural layout [128 b, 2 b_outer, 2048 fo]
            x0nat_r = x0np.tile([128, B // 128, F], f32r)
            for bt in range(B // 128):
                for fb in range(F // 128):
                    pt = tpool.tile([128, 128], f32, tag="tp")
                    nc.tensor.transpose(
                        pt, x0T[:, fb, bt * 128:(bt + 1) * 128].bitcast(f32), ident)
                    nc.any.tensor_copy(
                        out=x0nat_r[:, bt, fb * 128:(fb + 1) * 128], in_=pt)

            # ---------------- main loop over bs-chunks ----------------
            for c in range(NCHUNK):
                # -- build x_t chunk [128 fi, 16 fi_outer, NB bs] (f32r) --
                x_t = xtp.tile([128, F // 128, NB], f32r, tag="xt")
                for bsub in range(NB // 128):
                    r0 = c * NB + bsub * 128
                    for hf in range(2):
                        xnat = xnatp.tile([128, F // 2], f32, tag="xnat")
                        nc.sync.dma_start(
                            out=xnat,
                            in_=x_in[r0:r0 + 128, hf * (F // 2):(hf + 1) * (F // 2)])
                        for fj in range(F // 256):
                            fi = hf * (F // 256) + fj
                            pt = tpool.tile([128, 128], f32, tag="tp")
                            nc.tensor.transpose(
                                pt, xnat[:, fj * 128:(fj + 1) * 128], ident)
                            nc.any.tensor_copy(
                                out=x_t[:, fi, bsub * 128:(bsub + 1) * 128], in_=pt)

                # -- Wh chunk [128 fo, 16 fo_outer, NB bs] --
                wh = whp.tile([128, F // 128, NB], f32r, tag="wh")
                e_ps = []
                for _b2 in range(NB // 512):
                    e_ps_t = epool.tile([1, 512], f32, tag="ep", name=f"eps{rep}_{c}_{_b2}")
                    e_ps.append(e_ps_t)

                for fo in range(F // WCOLS):
                    wblk = wblkp.tile([128, F // 128, WCOLS], f32, tag="wblk")
                    nc.sync.dma_start(
                        out=wblk,
                        in_=w2_in.rearrange("(ko kp) n -> kp ko n", kp=128)[
                            :, :, fo * WCOLS:(fo + 1) * WCOLS],
                    )
                    wr = wrp.tile([128, F // 128, WCOLS], f32r, tag="wr")
                    nc.vector.tensor_copy(wr, wblk)
                    for b2 in range(NB // 512):
                        ps = ypool.tile([128, 512], f32, tag="yp")
                        for fi in range(F // 128):
                            nc.tensor.matmul(
                                ps,
                                wr[:, fi, :],
                                x_t[:, fi, b2 * 512:(b2 + 1) * 512],
                                start=(fi == 0),
                                stop=False,
                            )
                        # + broadcast_s(X0): selector matmul closes the group
                        nc.tensor.matmul(
                            ps,
                            x0nat_r[:, c, fo * 128:(fo + 1) * 128],
                            sel_r[:, b2 * 512:(b2 + 1) * 512],
                            start=False,
                            stop=True,
                        )
                        # leaky_relu: max(x, 0.3x); scale on ACT, max on DVE
                        t03 = t03p.tile([128, 512], f32, tag="t03")
                        nc.scalar.activation(t03, ps, AF.Copy, scale=ALPHA)
                        whs = wh[:, fo, b2 * 512:(b2 + 1) * 512]
                        nc.vector.tensor_tensor(
                            out=whs, in0=ps, in1=t03, op=OP.max)
                        # e += att_w[fo-block] . Wh  (PE matvec)
                        nc.tensor.matmul(
                            e_ps[b2],
                            attw_r[:, fo:fo + 1],
                            whs,
                            start=(fo == 0),
                            stop=(fo == F // 128 - 1),
                        )

                # -- softmax over s (groups of 8 along bs), scaled by 1/H --
                # ping-pong between two [1, NB] scratch tiles
                e_sb = esmp.tile([1, NB], f32, tag="esb", name=f"esb{rep}_{c}")
                for b2 in range(NB // 512):
                    nc.scalar.activation(
                        e_sb[:, b2 * 512:(b2 + 1) * 512], e_ps[b2], AF.Copy)
                work = esmp.tile([1, NB], f32, tag="work", name=f"work{rep}_{c}")
                e3 = e_sb.rearrange("p (b s) -> p b s", s=S)
                w3 = work.rearrange("p (b s) -> p b s", s=S)
                mx = esmp.tile([1, NB // S], f32, tag="mx", name=f"mx{rep}_{c}")
                nc.vector.reduce_max(out=mx, in_=e3, axis=mybir.AxisListType.X)
                nc.vector.tensor_tensor(
                    out=w3, in0=e3, in1=mx[:, :, None].to_broadcast((1, NB // S, S)),
                    op=OP.subtract)
                nc.scalar.activation(e_sb, work, AF.Exp)  # e_sb <- exp(work)
                sm = esmp.tile([1, NB // S], f32, tag="sm", name=f"sm{rep}_{c}")
                nc.vector.reduce_sum(out=sm, in_=e3, axis=mybir.AxisListType.X)
                rc = esmp.tile([1, NB // S], f32, tag="rc", name=f"rc{rep}_{c}")
                nc.vector.reciprocal(rc, sm)
                nc.vector.tensor_scalar_mul(rc, rc, 1.0 / H)
                attn_sb = work  # reuse: attn <- exp * rc
                a3 = w3
                nc.vector.tensor_tensor(
                    out=a3, in0=e3, in1=rc[:, :, None].to_broadcast((1, NB // S, S)),
                    op=OP.mult)

                # -- broadcast attn across partitions via a DRAM bounce --
                ab_full = abrsbp.tile([128, FSLICE // 128, NB], f32,
                                      tag="abrsb", name=f"abf{rep}_{c}")
                ab_sb = ab_full[:, 0, :]
                attn_dram = dpool.tile([1, NB], f32, tag="attn_dram")
                nc.gpsimd.dma_start(out=attn_dram[:, :], in_=attn_sb)
                attn_bc = bass.AP(
                    tensor=attn_dram.tensor,
                    offset=attn_dram.offset,
                    ap=[[0, 128]] + [list(p) for p in attn_dram[:, :].ap[1:]],
                )
                nc.gpsimd.dma_start(out=ab_sb, in_=attn_bc)

                # -- partial = attn/H * Wh -> DRAM --
                partial_c = dpool.tile([F, NB], f32, tag="partial")
                for fo in range(F // 128):
                    for b2 in range(NB // 512):
                        st = stagep.tile([128, 512], f32, tag="st")
                        nc.vector.tensor_tensor(
                            out=st,
                            in0=wh[:, fo, b2 * 512:(b2 + 1) * 512].bitcast(f32),
                            in1=ab_sb[:, b2 * 512:(b2 + 1) * 512],
                            op=OP.mult)
                        nc.sync.dma_start(
                            out=partial_c[fo * 128:(fo + 1) * 128,
                                          b2 * 512:(b2 + 1) * 512],
                            in_=st)

                # -- reduce over heads: ReduceScatter along f-axis --
                rs_c = dpool.tile([FSLICE, NB], f32, tag="rs")
                nc.gpsimd.collective_compute(
                    "ReduceScatter", OP.add,
                    replica_groups=[list(range(H))],
                    ins=[partial_c[:, :]], outs=[rs_c[:, :]])

                # -- finish: + bias_mean, transpose back, + residual, store --
                rsb = abrsbp.tile([128, FSLICE // 128, NB], f32,
                                  tag="abrsb", name=f"rsb{rep}_{c}")
                nc.gpsimd.dma_start(
                    out=rsb, in_=rs_c.rearrange("(o p) n -> p o n", p=128))
                for j in range(FSLICE // 128):
                    nc.scalar.activation(
                        rsb[:, j, :], rsb[:, j, :], AF.Identity,
                        bias=bm_sb[:, j:j + 1])
                for bsub in range(NB // 128):
                    r0 = c * NB + bsub * 128
                    for j in range(FSLICE // 128):
                        xrs = xrsp.tile([128, 128], f32, tag="xrs")
                        nc.sync.dma_start(
                            out=xrs,
                            in_=xres_in[r0:r0 + 128, j * 128:(j + 1) * 128])
                        pt = tpool.tile([128, 128], f32, tag="tp")
                        nc.tensor.transpose(
                            pt, rsb[:, j, bsub * 128:(bsub + 1) * 128], ident)
                        obl = outstp.tile([128, 128], f32, tag="obl")
                        nc.vector.tensor_tensor(
                            out=obl, in0=pt, in1=xrs, op=OP.add)
                        nc.sync.dma_start(
                            out=out_ext[r0:r0 + 128, j * 128:(j + 1) * 128],
                            in_=obl)


def _get_nc():
    if "nc" not in _cache:
        _cache["nc"] = _build()
    return _cache["nc"]


def _make_in_maps(inputs_dict):
    x = np.ascontiguousarray(
        np.asarray(inputs_dict["inputs"], dtype=np.float32).reshape(BS, F))
    W = np.asarray(inputs_dict["W"], dtype=np.float32)
    att_w = np.asarray(inputs_dict["att_w"], dtype=np.float32)
    bias = np.asarray(inputs_dict["bias"], dtype=np.float32)

    sel = np.repeat(np.eye(128, dtype=np.float32), S, axis=1)  # [128, 1024]
    bm_full = bias.mean(axis=0)  # [F]

    in_maps = []
    for i in range(H):
        in_maps.append({
            "w1": np.ascontiguousarray(W[i, :F, :]),
            "w2": np.ascontiguousarray(W[i, F:, :]),
            "x": x,
            "attw": np.ascontiguousarray(att_w[i]),
            "sel": sel,
            "xres": np.ascontiguousarray(x[:, FSLICE * i:FSLICE * (i + 1)]),
            "bm": np.ascontiguousarray(bm_full[FSLICE * i:FSLICE * (i + 1)]),
        })
    return in_maps


def kernel(inputs, W, att_w, bias):
    from concourse.bass_utils import run_bass_kernel_spmd

    nc = _get_nc()
    in_maps = _make_in_maps(
        {"inputs": inputs, "W": W, "att_w": att_w, "bias": bias})
    res = run_bass_kernel_spmd(nc, in_maps, list(range(H)))
    _cache["last_result"] = res

    out = np.concatenate([res.results[i]["out"] for i in range(H)], axis=1)
    return out.reshape(B, S, F)


# revision 19
# speedup vs baseline: 17.4135x; 17.4135x over previous
"""GATv2 self-attention kernel for 8 Trainium2 NeuronCores.

Sharding: one attention head per core (8 heads / 8 cores). Each core computes
its head's attn-weighted projection as a partial sum over heads, the cores
ReduceScatter the partials over the feature axis, and each core finishes its
256-column feature slice (bias-mean + residual) and returns it; the host
concatenates the 8 slices.

Math per head h (reference):
  X = inputs.reshape(B*S, F); x0 = rows of X with s == 0
  Wh = leaky_relu(X @ W2h + broadcast_s(x0 @ W1h))      [B*S, F]
  e  = Wh @ att_w[h]; attn = softmax_s(e)
  out = sum_h (attn * Wh)/H + mean_h(bias) + X

All matmuls run in float32r (TF32-like, 11-bit mantissa) on the PE array.
X is transposed on-chip via PE transpose-mode matmuls. The broadcast x0@W1
term is accumulated into the same PSUM group as the X@W2 tiles through a
0/1 selector matmul, so no elementwise broadcast-add pass is needed.
"""

import sys
import os
import numpy as np

sys.path.insert(0, "/opt/trn_rl_repo")

B, S, F, H = 256, 8, 2048, 8
BS = B * S            # 2048
NB = 1024             # bs-chunk size (2 chunks)
NCHUNK = BS // NB     # 2
FSLICE = F // H       # 256 output feature columns per core
ALPHA = 0.3
USE_PRELU = True

_cache = {}


def _build(reps=1):
    import concourse.bacc as bacc
    import concourse.mybir as mybir
    import concourse.tile as tile
    import concourse.bass as bass
    from concourse.masks import make_identity

    f32 = mybir.dt.float32
    f32r = mybir.dt.float32r
    AF = mybir.ActivationFunctionType
    OP = mybir.AluOpType

    nc = bacc.Bacc(num_devices=H)

    w1_in = nc.declare_dram_parameter(
        "w1t", [F // 128, 128, F // 128, 128], f32, isOutput=False)
    w2_in = nc.declare_dram_parameter(
        "w2t", [F // 128, 128, F // 128, 128], f32, isOutput=False)
    x_in = nc.declare_dram_parameter("x", [BS, F], f32, isOutput=False)
    attw_in = nc.declare_dram_parameter("attw", [F], f32, isOutput=False)
    sel_in = nc.declare_dram_parameter("sel", [128, NB], f32, isOutput=False)
    xres_in = nc.declare_dram_parameter("xres", [BS, FSLICE], f32, isOutput=False)
    bm_in = nc.declare_dram_parameter("bm", [FSLICE], f32, isOutput=False)
    out_ext = nc.declare_dram_parameter("out", [BS, FSLICE], f32, isOutput=True)

    from contextlib import ExitStack
    with tile.TileContext(nc) as tc:
        with ExitStack() as ctx:
            consts = ctx.enter_context(tc.tile_pool(name="consts", bufs=1))
            xtp = ctx.enter_context(tc.tile_pool(name="xtp", bufs=1))
            whp = ctx.enter_context(tc.tile_pool(name="whp", bufs=1))
            wblkp = ctx.enter_context(tc.tile_pool(name="wblk", bufs=1))
            wrp = ctx.enter_context(tc.tile_pool(name="wrp", bufs=2))
            x0np = ctx.enter_context(tc.tile_pool(name="x0n", bufs=1))
            xnatp = ctx.enter_context(tc.tile_pool(name="xnat", bufs=2))
            t03p = ctx.enter_context(tc.tile_pool(name="t03", bufs=2))
            stagep = ctx.enter_context(tc.tile_pool(name="stage", bufs=2))
            esmp = ctx.enter_context(tc.tile_pool(name="esm", bufs=1))
            abrsbp = ctx.enter_context(tc.tile_pool(name="abrsb", bufs=1))
            xrsp = ctx.enter_context(tc.tile_pool(name="xrs", bufs=1))
            outstp = ctx.enter_context(tc.tile_pool(name="outst", bufs=1))
            ypool = ctx.enter_context(tc.tile_pool(name="ypool", bufs=4, space="PSUM"))
            epool = ctx.enter_context(tc.tile_pool(name="epool", bufs=2, space="PSUM"))
            tpool = ctx.enter_context(tc.tile_pool(name="tpool", bufs=2, space="PSUM"))
            dpool = ctx.enter_context(tc.tile_pool(name="dram", bufs=2, space="DRAM"))

            # ---------------- constants ----------------
            ident = consts.tile([128, 128], f32)
            make_identity(nc, ident)

            attw_f = consts.tile([128, F // 128], f32)
            nc.sync.dma_start(out=attw_f, in_=attw_in.rearrange("(o p) -> p o", p=128))
            attw_r = consts.tile([128, F // 128], f32r)
            nc.vector.tensor_copy(attw_r, attw_f)

            sel_r = consts.tile([128, NB], f32r)
            for half in range(NB // 512):
                sh = stagep.tile([128, 512], f32, tag="st")
                nc.sync.dma_start(out=sh, in_=sel_in[:, half * 512:(half + 1) * 512])
                nc.vector.tensor_copy(sel_r[:, half * 512:(half + 1) * 512], sh)

            al_sb = consts.tile([128, 1], f32)
            nc.vector.memset(al_sb, ALPHA)

            bm_sb = consts.tile([128, FSLICE // 128], f32)
            nc.sync.dma_start(out=bm_sb, in_=bm_in.rearrange("(o p) -> p o", p=128))

            # view of x grouped by (b, s)
            x_bsf = x_in.rearrange("(b s) f -> b s f", s=S)

            for _rep in range(reps):
                _run_body(nc, tc, mybir, bass, f32, f32r, AF, OP,
                          make_identity, ident, attw_r, sel_r, bm_sb, al_sb, x_bsf,
                          w1_in, w2_in, x_in, xres_in, out_ext,
                          consts, xtp, whp, wblkp, wrp, x0np, xnatp, t03p,
                          stagep, esmp, abrsbp, xrsp, outstp,
                          ypool, epool, tpool, dpool, _rep)

    nc.compile()
    return nc


def _run_body(nc, tc, mybir, bass, f32, f32r, AF, OP,
              make_identity, ident, attw_r, sel_r, bm_sb, al_sb, x_bsf,
              w1_in, w2_in, x_in, xres_in, out_ext,
              consts, xtp, whp, wblkp, wrp, x0np, xnatp, t03p,
              stagep, esmp, abrsbp, xrsp, outstp,
              ypool, epool, tpool, dpool, rep):
    if True:
        if True:
            # ---------------- prologue: X0 = x0 @ W1 (natural [b, f] layout) --------
            # x0t: [128 fi, 16 fi_outer, 256 b] carved out of the Wh buffer slot
            x0t_full = whp.tile([128, F // 128, NB], f32r, tag="wh")
            x0t = x0t_full[:, :, :B]
            for bt in range(B // 128):
                x0nat = xnatp.tile([128, F], f32, tag="xnat")
                nc.sync.dma_start(
                    out=x0nat, in_=x_bsf[bt * 128:(bt + 1) * 128, 0, :])
                for fi in range(F // 128):
                    pt = tpool.tile([128, 128], f32, tag="tp")
                    nc.tensor.transpose(
                        pt, x0nat[:, fi * 128:(fi + 1) * 128], ident)
                    nc.any.tensor_copy(
                        out=x0t[:, fi, bt * 128:(bt + 1) * 128], in_=pt)

            # X0 in transposed layout first: X0_T [128 fo, 16 fo_outer, 256 b],
            # stored in spare columns of the same wh-tag slot as x0t.
            x0T = x0t_full[:, :, B:2 * B]
            WCOLS = 128  # fo columns per W-block load
            for fb in range(F // WCOLS):
                wblk = wblkp.tile([128, F // 128, WCOLS], f32, tag="wblk")
                nc.sync.dma_start(out=wblk, in_=w1_in[fb])
                wr = wrp.tile([128, F // 128, WCOLS], f32r, tag="wr")
                nc.vector.tensor_copy(wr, wblk)
                ps_full = ypool.tile([128, 512], f32, tag="yp")
                ps = ps_full[:, :B]
                for fi in range(F // 128):
                    nc.tensor.matmul(
                        ps,
                        wr[:, fi, :],
                        x0t[:, fi, :],
                        start=(fi == 0),
                        stop=(fi == F // 128 - 1),
                    )
                nc.any.tensor_copy(out=x0T[:, fb, :], in_=ps)

            # transpose X0_T back to natural layout [128 b, 2 b_outer, 2048 fo]
            x0nat_r = x0np.tile([128, B // 128, F], f32r)
            for bt in range(B // 128):
                for fb in range(F // 128):
                    pt = tpool.tile([128, 128], f32, tag="tp")
                    nc.tensor.transpose(
                        pt, x0T[:, fb, bt * 128:(bt + 1) * 128].bitcast(f32), ident)
                    nc.any.tensor_copy(
                        out=x0nat_r[:, bt, fb * 128:(fb + 1) * 128], in_=pt)

            # ---------------- main loop over bs-chunks ----------------
            for c in range(NCHUNK):
                # -- build x_t chunk [128 fi, 16 fi_outer, NB bs] (f32r) --
                x_t = xtp.tile([128, F // 128, NB], f32r, tag="xt")
                for bsub in range(NB // 128):
                    r0 = c * NB + bsub * 128
                    xnat = xnatp.tile([128, F], f32, tag="xnat")
                    nc.sync.dma_start(out=xnat, in_=x_in[r0:r0 + 128, :])
                    for fi in range(F // 128):
                        pt = tpool.tile([128, 128], f32, tag="tp")
                        nc.tensor.transpose(
                            pt, xnat[:, fi * 128:(fi + 1) * 128], ident)
                        nc.any.tensor_copy(
                            out=x_t[:, fi, bsub * 128:(bsub + 1) * 128], in_=pt)

                # -- Wh chunk [128 fo, 16 fo_outer, NB bs] --
                wh = whp.tile([128, F // 128, NB], f32r, tag="wh")
                e_ps = []
                for _b2 in range(NB // 512):
                    e_ps_t = epool.tile([1, 512], f32, tag="ep", name=f"eps{rep}_{c}_{_b2}")
                    e_ps.append(e_ps_t)

                for fo in range(F // WCOLS):
                    wblk = wblkp.tile([128, F // 128, WCOLS], f32, tag="wblk")
                    nc.sync.dma_start(out=wblk, in_=w2_in[fo])
                    wr = wrp.tile([128, F // 128, WCOLS], f32r, tag="wr")
                    nc.vector.tensor_copy(wr, wblk)
                    for b2 in range(NB // 512):
                        ps = ypool.tile([128, 512], f32, tag="yp")
                        for fi in range(F // 128):
                            nc.tensor.matmul(
                                ps,
                                wr[:, fi, :],
                                x_t[:, fi, b2 * 512:(b2 + 1) * 512],
                                start=(fi == 0),
                                stop=False,
                            )
                        # + broadcast_s(X0): selector matmul closes the group
                        nc.tensor.matmul(
                            ps,
                            x0nat_r[:, c, fo * 128:(fo + 1) * 128],
                            sel_r[:, b2 * 512:(b2 + 1) * 512],
                            start=False,
                            stop=True,
                        )
                        # leaky_relu: max(x, 0.3x)
                        whs = wh[:, fo, b2 * 512:(b2 + 1) * 512]
                        if USE_PRELU:
                            nc.scalar.activation(whs, ps, AF.Prelu, alpha=al_sb[:, :])
                        else:
                            t03 = t03p.tile([128, 512], f32, tag="t03")
                            nc.scalar.activation(t03, ps, AF.Copy, scale=ALPHA)
                            nc.vector.tensor_tensor(
                                out=whs, in0=ps, in1=t03, op=OP.max)
                        # e += att_w[fo-block] . Wh  (PE matvec)
                        nc.tensor.matmul(
                            e_ps[b2],
                            attw_r[:, fo:fo + 1],
                            whs,
                            start=(fo == 0),
                            stop=(fo == F // 128 - 1),
                        )

                # -- softmax over s (groups of 8 along bs), scaled by 1/H --
                # processed per 512-wide half; broadcast via DRAM bounce
                ab_full = abrsbp.tile([128, FSLICE // 128, NB], f32,
                                      tag="abrsb", name=f"abf{rep}_{c}")
                ab_sb = ab_full[:, 0, :]
                NG = 512 // S  # softmax groups per half
                for b2 in range(NB // 512):
                    e_sb = esmp.tile([1, 512], f32, tag="esb",
                                     name=f"esb{rep}_{c}_{b2}")
                    nc.scalar.activation(e_sb, e_ps[b2], AF.Copy)
                    work = esmp.tile([1, 512], f32, tag="work",
                                     name=f"work{rep}_{c}_{b2}")
                    e3 = e_sb.rearrange("p (b s) -> p b s", s=S)
                    w3 = work.rearrange("p (b s) -> p b s", s=S)
                    mx = esmp.tile([1, NG], f32, tag="mx", name=f"mx{rep}_{c}_{b2}")
                    nc.vector.reduce_max(out=mx, in_=e3, axis=mybir.AxisListType.X)
                    nc.vector.tensor_tensor(
                        out=w3, in0=e3,
                        in1=mx[:, :, None].to_broadcast((1, NG, S)),
                        op=OP.subtract)
                    nc.scalar.activation(e_sb, work, AF.Exp)  # e_sb <- exp(work)
                    sm = esmp.tile([1, NG], f32, tag="sm", name=f"sm{rep}_{c}_{b2}")
                    nc.vector.reduce_sum(out=sm, in_=e3, axis=mybir.AxisListType.X)
                    rc = esmp.tile([1, NG], f32, tag="rc", name=f"rc{rep}_{c}_{b2}")
                    nc.vector.reciprocal(rc, sm)
                    nc.vector.tensor_scalar_mul(rc, rc, 1.0 / H)
                    attn_sb = work  # reuse: attn <- exp * rc
                    nc.vector.tensor_tensor(
                        out=w3, in0=e3,
                        in1=rc[:, :, None].to_broadcast((1, NG, S)),
                        op=OP.mult)
                    attn_dram = dpool.tile([1, 512], f32, tag="attn_dram")
                    nc.gpsimd.dma_start(out=attn_dram[:, :], in_=attn_sb)
                    attn_bc = bass.AP(
                        tensor=attn_dram.tensor,
                        offset=attn_dram.offset,
                        ap=[[0, 128]] + [list(p) for p in attn_dram[:, :].ap[1:]],
                    )
                    nc.gpsimd.dma_start(
                        out=ab_sb[:, b2 * 512:(b2 + 1) * 512], in_=attn_bc)

                # -- partial = attn/H * Wh -> DRAM --
                partial_c = dpool.tile([F, NB], f32, tag="partial")
                for fo in range(F // 128):
                    for b2 in range(NB // 512):
                        st = stagep.tile([128, 512], f32, tag="st")
                        nc.vector.tensor_tensor(
                            out=st,
                            in0=wh[:, fo, b2 * 512:(b2 + 1) * 512].bitcast(f32),
                            in1=ab_sb[:, b2 * 512:(b2 + 1) * 512],
                            op=OP.mult)
                        nc.gpsimd.dma_start(
                            out=partial_c[fo * 128:(fo + 1) * 128,
                                          b2 * 512:(b2 + 1) * 512],
                            in_=st)

                # -- reduce over heads: ReduceScatter along f-axis --
                rs_c = dpool.tile([FSLICE, NB], f32, tag="rs")
                nc.gpsimd.collective_compute(
                    "ReduceScatter", OP.add,
                    replica_groups=[list(range(H))],
                    ins=[partial_c[:, :]], outs=[rs_c[:, :]])

                # -- finish: + bias_mean, transpose back, + residual, store --
                rsb = abrsbp.tile([128, FSLICE // 128, NB], f32,
                                  tag="abrsb", name=f"rsb{rep}_{c}")
                nc.gpsimd.dma_start(
                    out=rsb, in_=rs_c.rearrange("(o p) n -> p o n", p=128))
                for j in range(FSLICE // 128):
                    nc.scalar.activation(
                        rsb[:, j, :], rsb[:, j, :], AF.Identity,
                        bias=bm_sb[:, j:j + 1])
                for bsub in range(NB // 128):
                    r0 = c * NB + bsub * 128
                    xrs = xrsp.tile([128, FSLICE], f32, tag="xrs")
                    nc.sync.dma_start(out=xrs, in_=xres_in[r0:r0 + 128, :])
                    obl = outstp.tile([128, FSLICE // 128, 128], f32, tag="obl")
                    for j in range(FSLICE // 128):
                        pt = tpool.tile([128, 128], f32, tag="tp")
                        nc.tensor.transpose(
                            pt, rsb[:, j, bsub * 128:(bsub + 1) * 128], ident)
                        nc.vector.tensor_tensor(
                            out=obl[:, j, :], in0=pt,
                            in1=xrs[:, j * 128:(j + 1) * 128], op=OP.add)
                    nc.gpsimd.dma_start(
                        out=out_ext[r0:r0 + 128, :],
                        in_=obl.rearrange("p a b -> p (a b)"))


def _get_nc():
    if "nc" not in _cache:
        _cache["nc"] = _build()
    return _cache["nc"]


def _make_in_maps(inputs_dict):
    x = np.ascontiguousarray(
        np.asarray(inputs_dict["inputs"], dtype=np.float32).reshape(BS, F))
    W = np.asarray(inputs_dict["W"], dtype=np.float32)
    att_w = np.asarray(inputs_dict["att_w"], dtype=np.float32)
    bias = np.asarray(inputs_dict["bias"], dtype=np.float32)

    sel = np.repeat(np.eye(128, dtype=np.float32), S, axis=1)  # [128, 1024]
    bm_full = bias.mean(axis=0)  # [F]

    def tile_w(w):
        # [F, F] -> [fo_blk, kp, ko, n] with fi = ko*128 + kp, fo = fo_blk*128 + n
        return np.ascontiguousarray(
            w.reshape(F // 128, 128, F // 128, 128).transpose(2, 1, 0, 3))

    in_maps = []
    for i in range(H):
        in_maps.append({
            "w1t": tile_w(W[i, :F, :]),
            "w2t": tile_w(W[i, F:, :]),
            "x": x,
            "attw": np.ascontiguousarray(att_w[i]),
            "sel": sel,
            "xres": np.ascontiguousarray(x[:, FSLICE * i:FSLICE * (i + 1)]),
            "bm": np.ascontiguousarray(bm_full[FSLICE * i:FSLICE * (i + 1)]),
        })
    return in_maps


def kernel(inputs, W, att_w, bias):
    from concourse.bass_utils import run_bass_kernel_spmd

    nc = _get_nc()
    in_maps = _make_in_maps(
        {"inputs": inputs, "W": W, "att_w": att_w, "bias": bias})
    res = run_bass_kernel_spmd(nc, in_maps, list(range(H)))
    _cache["last_result"] = res

    out = np.concatenate([res.results[i]["out"] for i in range(H)], axis=1)
    return out.reshape(B, S, F)


# revision 25
# speedup vs baseline: 24.0629x; 1.3818x over previous
"""GATv2 self-attention kernel for 8 Trainium2 NeuronCores.

Sharding: one attention head per core (8 heads / 8 cores). Each core computes
its head's attn-weighted projection as a partial sum over heads, the cores
ReduceScatter the partials over the feature axis, and each core finishes its
256-column feature slice (bias-mean + residual) and returns it; the host
concatenates the 8 slices.

Math per head h (reference):
  X = inputs.reshape(B*S, F); x0 = rows of X with s == 0
  Wh = leaky_relu(X @ W2h + broadcast_s(x0 @ W1h))      [B*S, F]
  e  = Wh @ att_w[h]; attn = softmax_s(e)
  out = sum_h (attn * Wh)/H + mean_h(bias) + X

All matmuls run in float32r (TF32-like, 11-bit mantissa) on the PE array.
X is transposed on-chip via PE transpose-mode matmuls. The broadcast x0@W1
term is accumulated into the same PSUM group as the X@W2 tiles through a
0/1 selector matmul, so no elementwise broadcast-add pass is needed.
"""

import sys
import os
import numpy as np

sys.path.insert(0, "/opt/trn_rl_repo")

B, S, F, H = 256, 8, 2048, 8
BS = B * S            # 2048
NB = 1024             # bs-chunk size (2 chunks)
NCHUNK = BS // NB     # 2
FSLICE = F // H       # 256 output feature columns per core
ALPHA = 0.3
USE_PRELU = True

_cache = {}


def _build(reps=1):
    import concourse.bacc as bacc
    import concourse.mybir as mybir
    import concourse.tile as tile
    import concourse.bass as bass
    from concourse.masks import make_identity

    f32 = mybir.dt.float32
    f32r = mybir.dt.float32r
    AF = mybir.ActivationFunctionType
    OP = mybir.AluOpType

    nc = bacc.Bacc(num_devices=H)

    w1_in = nc.declare_dram_parameter(
        "w1t", [F // 256, 128, 2, F // 128, 128], f32, isOutput=False)
    w2_in = nc.declare_dram_parameter(
        "w2t", [F // 256, 128, 2, F // 128, 128], f32, isOutput=False)
    x_in = nc.declare_dram_parameter("x", [BS, F], f32, isOutput=False)
    attw_in = nc.declare_dram_parameter("attw", [F], f32, isOutput=False)
    sel_in = nc.declare_dram_parameter("sel", [128, NB], f32, isOutput=False)
    xres_in = nc.declare_dram_parameter("xres", [BS, FSLICE], f32, isOutput=False)
    bm_in = nc.declare_dram_parameter("bm", [FSLICE], f32, isOutput=False)
    out_ext = nc.declare_dram_parameter("out", [BS, FSLICE], f32, isOutput=True)

    from contextlib import ExitStack
    with tile.TileContext(nc) as tc:
        with ExitStack() as ctx:
            consts = ctx.enter_context(tc.tile_pool(name="consts", bufs=1))
            xtp = ctx.enter_context(tc.tile_pool(name="xtp", bufs=1))
            whp = ctx.enter_context(tc.tile_pool(name="whp", bufs=1))
            wblkp = ctx.enter_context(tc.tile_pool(name="wblk", bufs=1))
            wrp = ctx.enter_context(tc.tile_pool(name="wrp", bufs=2))
            x0cp = ctx.enter_context(tc.tile_pool(name="x0c", bufs=1))
            xnatp = ctx.enter_context(tc.tile_pool(name="xnat", bufs=2))
            t03p = ctx.enter_context(tc.tile_pool(name="t03", bufs=2))
            esmp = ctx.enter_context(tc.tile_pool(name="esm", bufs=1))
            abrsbp = ctx.enter_context(tc.tile_pool(name="abrsb", bufs=1))
            xrsp = ctx.enter_context(tc.tile_pool(name="xrs", bufs=1))
            outstp = ctx.enter_context(tc.tile_pool(name="outst", bufs=1))
            ypool = ctx.enter_context(tc.tile_pool(name="ypool", bufs=4, space="PSUM"))
            epool = ctx.enter_context(tc.tile_pool(name="epool", bufs=2, space="PSUM"))
            tpool = ctx.enter_context(tc.tile_pool(name="tpool", bufs=2, space="PSUM"))
            dpool = ctx.enter_context(tc.tile_pool(name="dram", bufs=2, space="DRAM"))

            # ---------------- constants ----------------
            ident = consts.tile([128, 128], f32)
            make_identity(nc, ident)

            attw_f = consts.tile([128, F // 128], f32)
            nc.sync.dma_start(out=attw_f, in_=attw_in.rearrange("(o p) -> p o", p=128))
            attw_r = consts.tile([128, F // 128], f32r)
            nc.vector.tensor_copy(attw_r, attw_f)

            sel_r = consts.tile([128, NB], f32r)
            sel_stg = xnatp.tile([128, F], f32, tag="xnat")
            nc.sync.dma_start(out=sel_stg[:, :NB], in_=sel_in[:, :])
            nc.vector.tensor_copy(sel_r, sel_stg[:, :NB])

            al_sb = consts.tile([128, 1], f32)
            nc.vector.memset(al_sb, ALPHA)

            bm_sb = consts.tile([128, FSLICE // 128], f32)
            nc.sync.dma_start(out=bm_sb, in_=bm_in.rearrange("(o p) -> p o", p=128))

            # view of x grouped by (b, s)
            x_bsf = x_in.rearrange("(b s) f -> b s f", s=S)

            for _rep in range(reps):
                _run_body(nc, tc, mybir, bass, f32, f32r, AF, OP,
                          make_identity, ident, attw_r, sel_r, bm_sb, al_sb, x_bsf,
                          w1_in, w2_in, x_in, xres_in, out_ext,
                          consts, xtp, whp, wblkp, wrp, x0cp, xnatp, t03p,
                          esmp, abrsbp, xrsp, outstp,
                          ypool, epool, tpool, dpool, _rep)

    nc.compile()
    return nc


def _run_body(nc, tc, mybir, bass, f32, f32r, AF, OP,
              make_identity, ident, attw_r, sel_r, bm_sb, al_sb, x_bsf,
              w1_in, w2_in, x_in, xres_in, out_ext,
              consts, xtp, whp, wblkp, wrp, x0cp, xnatp, t03p,
              esmp, abrsbp, xrsp, outstp,
              ypool, epool, tpool, dpool, rep):
    NFB = F // 128   # 16 fo/fi blocks
    # ---------------- prologue: X0 = x0 @ W1 ----------------
    # x0t: [128 fi, 16 fi_outer, 256 b] carved out of the Wh buffer slot
    x0t_full = whp.tile([128, NFB, NB], f32r, tag="wh")
    x0t = x0t_full[:, :, :B]
    for bt in range(B // 128):
        x0nat = xnatp.tile([128, F], f32, tag="xnat")
        nc.sync.dma_start(
            out=x0nat, in_=x_bsf[bt * 128:(bt + 1) * 128, 0, :])
        for fi in range(NFB):
            pt = tpool.tile([128, 128], f32, tag="tp")
            nc.tensor.transpose(
                pt, x0nat[:, fi * 128:(fi + 1) * 128], ident)
            nc.any.tensor_copy(
                out=x0t[:, fi, bt * 128:(bt + 1) * 128], in_=pt)

    # X0 in transposed layout first: X0_T [128 fo, 16 fo_outer, 256 b]
    x0T = x0t_full[:, :, B:2 * B]
    for pair in range(NFB // 2):
        wblk = wblkp.tile([128, 2, NFB, 128], f32, tag="wblk")
        nc.sync.dma_start(out=wblk, in_=w1_in[pair])
        for half in range(2):
            fb = pair * 2 + half
            wr = wrp.tile([128, NFB, 128], f32r, tag="wr")
            nc.vector.tensor_copy(wr, wblk[:, half])
            ps_full = ypool.tile([128, 512], f32, tag="yp")
            ps = ps_full[:, :B]
            for fi in range(NFB):
                nc.tensor.matmul(
                    ps, wr[:, fi, :], x0t[:, fi, :],
                    start=(fi == 0), stop=(fi == NFB - 1))
            nc.any.tensor_copy(out=x0T[:, fb, :], in_=ps)

    # transpose X0_T back to natural layout, stage to DRAM (fp32)
    x0_dram = dpool.tile([B // 128, 128, F], f32, tag="x0dram")
    for bt in range(B // 128):
        x0stg = xnatp.tile([128, F], f32, tag="xnat")
        for fb in range(NFB):
            pt = tpool.tile([128, 128], f32, tag="tp")
            nc.tensor.transpose(
                pt, x0T[:, fb, bt * 128:(bt + 1) * 128].bitcast(f32), ident)
            nc.any.tensor_copy(
                out=x0stg[:, fb * 128:(fb + 1) * 128], in_=pt)
        nc.sync.dma_start(out=x0_dram[bt], in_=x0stg)

    # ---------------- main loop over bs-chunks ----------------
    rs_list = []
    for c in range(NCHUNK):
        # X0 rows for this chunk: load + round to f32r
        x0ld = xnatp.tile([128, F], f32, tag="xnat")
        nc.sync.dma_start(out=x0ld, in_=x0_dram[c])
        x0c_r = x0cp.tile([128, F], f32r, tag="x0c", name=f"x0c{rep}_{c}")
        nc.vector.tensor_copy(x0c_r, x0ld)

        # -- build x_t chunk [128 fi, 16 fi_outer, NB bs] (f32r) --
        x_t = xtp.tile([128, NFB, NB], f32r, tag="xt")
        for bsub in range(NB // 128):
            r0 = c * NB + bsub * 128
            xnat = xnatp.tile([128, F], f32, tag="xnat")
            eng = nc.sync if bsub % 2 == 0 else nc.gpsimd
            eng.dma_start(out=xnat, in_=x_in[r0:r0 + 128, :])
            for fi in range(NFB):
                pt = tpool.tile([128, 128], f32, tag="tp")
                nc.tensor.transpose(
                    pt, xnat[:, fi * 128:(fi + 1) * 128], ident)
                nc.any.tensor_copy(
                    out=x_t[:, fi, bsub * 128:(bsub + 1) * 128], in_=pt)

        # -- Wh chunk [128 fo, 16 fo_outer, NB bs] --
        wh = whp.tile([128, NFB, NB], f32r, tag="wh")
        e_ps = []
        for _b2 in range(NB // 512):
            e_ps_t = epool.tile([1, 512], f32, tag="ep",
                                name=f"eps{rep}_{c}_{_b2}")
            e_ps.append(e_ps_t)

        pending_e = []  # lag e-matvecs one fo-block so PE never waits on ACT
        for pair in range(NFB // 2):
            wblk = wblkp.tile([128, 2, NFB, 128], f32, tag="wblk")
            nc.sync.dma_start(out=wblk, in_=w2_in[pair])
            for half in range(2):
                fo = pair * 2 + half
                wr = wrp.tile([128, NFB, 128], f32r, tag="wr")
                nc.vector.tensor_copy(wr, wblk[:, half])
                for b2 in range(NB // 512):
                    ps = ypool.tile([128, 512], f32, tag="yp")
                    for fi in range(NFB):
                        nc.tensor.matmul(
                            ps, wr[:, fi, :],
                            x_t[:, fi, b2 * 512:(b2 + 1) * 512],
                            start=(fi == 0), stop=False)
                    # + broadcast_s(X0): selector matmul closes the group
                    nc.tensor.matmul(
                        ps, x0c_r[:, fo * 128:(fo + 1) * 128],
                        sel_r[:, b2 * 512:(b2 + 1) * 512],
                        start=False, stop=True)
                    # leaky_relu on ACT (exact Prelu), writes f32r Wh
                    whs = wh[:, fo, b2 * 512:(b2 + 1) * 512]
                    if USE_PRELU:
                        nc.scalar.activation(whs, ps, AF.Prelu,
                                             alpha=al_sb[:, :])
                    else:
                        t03 = t03p.tile([128, 512], f32, tag="t03")
                        nc.scalar.activation(t03, ps, AF.Copy, scale=ALPHA)
                        nc.vector.tensor_tensor(
                            out=whs, in0=ps, in1=t03, op=OP.max)
                    pending_e.append((fo, b2, whs))
                # emit lagged e-matvecs (previous fo block)
                while len(pending_e) > NB // 512:
                    efo, eb2, ewhs = pending_e.pop(0)
                    nc.tensor.matmul(
                        e_ps[eb2], attw_r[:, efo:efo + 1], ewhs,
                        start=(efo == 0), stop=(efo == NFB - 1))
        for efo, eb2, ewhs in pending_e:
            nc.tensor.matmul(
                e_ps[eb2], attw_r[:, efo:efo + 1], ewhs,
                start=(efo == 0), stop=(efo == NFB - 1))

        # -- softmax over s (groups of 8 along bs), scaled by 1/H --
        ab_full = abrsbp.tile([128, FSLICE // 128, NB], f32,
                              tag="abrsb", name=f"abf{rep}_{c}")
        ab_sb = ab_full[:, 0, :]
        NG = 512 // S
        for b2 in range(NB // 512):
            e_sb = esmp.tile([1, 512], f32, tag="esb",
                             name=f"esb{rep}_{c}_{b2}")
            nc.scalar.activation(e_sb, e_ps[b2], AF.Copy)
            work = esmp.tile([1, 512], f32, tag="work",
                             name=f"work{rep}_{c}_{b2}")
            e3 = e_sb.rearrange("p (b s) -> p b s", s=S)
            w3 = work.rearrange("p (b s) -> p b s", s=S)
            mx = esmp.tile([1, NG], f32, tag="mx", name=f"mx{rep}_{c}_{b2}")
            nc.vector.reduce_max(out=mx, in_=e3, axis=mybir.AxisListType.X)
            nc.vector.tensor_tensor(
                out=w3, in0=e3, in1=mx[:, :, None].to_broadcast((1, NG, S)),
                op=OP.subtract)
            nc.scalar.activation(e_sb, work, AF.Exp)
            sm = esmp.tile([1, NG], f32, tag="sm", name=f"sm{rep}_{c}_{b2}")
            nc.vector.reduce_sum(out=sm, in_=e3, axis=mybir.AxisListType.X)
            rc = esmp.tile([1, NG], f32, tag="rc", name=f"rc{rep}_{c}_{b2}")
            nc.vector.reciprocal(rc, sm)
            nc.vector.tensor_scalar_mul(rc, rc, 1.0 / H)
            attn_sb = work
            nc.vector.tensor_tensor(
                out=w3, in0=e3, in1=rc[:, :, None].to_broadcast((1, NG, S)),
                op=OP.mult)
            attn_dram = dpool.tile([1, 512], f32, tag="attn_dram")
            nc.gpsimd.dma_start(out=attn_dram[:, :], in_=attn_sb)
            attn_bc = bass.AP(
                tensor=attn_dram.tensor,
                offset=attn_dram.offset,
                ap=[[0, 128]] + [list(p) for p in attn_dram[:, :].ap[1:]],
            )
            nc.gpsimd.dma_start(
                out=ab_sb[:, b2 * 512:(b2 + 1) * 512], in_=attn_bc)

        # -- partial = attn/H * Wh, in place, then 2 bulk DMAs --
        partial_c = dpool.tile([F, NB], f32, tag="partial")
        pview = partial_c.rearrange("(o p) n -> p o n", p=128)
        for fo in range(NFB):
            whs = wh[:, fo, :]
            nc.vector.tensor_tensor(
                out=whs, in0=whs.bitcast(f32), in1=ab_sb, op=OP.mult)
        for hh in range(2):
            nc.gpsimd.dma_start(
                out=pview[:, hh * 8:(hh + 1) * 8, :],
                in_=wh[:, hh * 8:(hh + 1) * 8, :].bitcast(f32))

        # -- reduce over heads: ReduceScatter along f-axis --
        rs_c = dpool.tile([FSLICE, NB], f32, tag="rs", name=f"rs{rep}_{c}")
        nc.gpsimd.collective_compute(
            "ReduceScatter", OP.add,
            replica_groups=[list(range(H))],
            ins=[partial_c[:, :]], outs=[rs_c[:, :]])
        rs_list.append(rs_c)

    # finish phase after all chunks (overlaps trailing collectives)
    for c in range(NCHUNK):
        _finish_chunk(nc, tc, mybir, bass, f32, f32r, AF, OP, ident, bm_sb,
                      xres_in, out_ext, abrsbp, xrsp, outstp, tpool,
                      rs_list[c], c, rep)


def _get_nc():
    if "nc" not in _cache:
        _cache["nc"] = _build()
    return _cache["nc"]


def _make_in_maps(inputs_dict):
    x = np.ascontiguousarray(
        np.asarray(inputs_dict["inputs"], dtype=np.float32).reshape(BS, F))
    W = np.asarray(inputs_dict["W"], dtype=np.float32)
    att_w = np.asarray(inputs_dict["att_w"], dtype=np.float32)
    bias = np.asarray(inputs_dict["bias"], dtype=np.float32)

    sel = np.repeat(np.eye(128, dtype=np.float32), S, axis=1)  # [128, 1024]
    bm_full = bias.mean(axis=0)  # [F]

    def tile_w(w):
        # [F, F] -> [pair, kp, b, ko, n]; fi = ko*128+kp, fo = pair*256+b*128+n
        t = w.reshape(F // 128, 128, F // 256, 2, 128)  # [ko, kp, pair, b, n]
        return np.ascontiguousarray(t.transpose(2, 1, 3, 0, 4))

    in_maps = []
    for i in range(H):
        in_maps.append({
            "w1t": tile_w(W[i, :F, :]),
            "w2t": tile_w(W[i, F:, :]),
            "x": x,
            "attw": np.ascontiguousarray(att_w[i]),
            "sel": sel,
            "xres": np.ascontiguousarray(x[:, FSLICE * i:FSLICE * (i + 1)]),
            "bm": np.ascontiguousarray(bm_full[FSLICE * i:FSLICE * (i + 1)]),
        })
    return in_maps


def kernel(inputs, W, att_w, bias):
    from concourse.bass_utils import run_bass_kernel_spmd

    nc = _get_nc()
    in_maps = _make_in_maps(
        {"inputs": inputs, "W": W, "att_w": att_w, "bias": bias})
    res = run_bass_kernel_spmd(nc, in_maps, list(range(H)))
    _cache["last_result"] = res

    out = np.concatenate([res.results[i]["out"] for i in range(H)], axis=1)
    return out.reshape(B, S, F)


def _finish_chunk(nc, tc, mybir, bass, f32, f32r, AF, OP, ident, bm_sb,
                  xres_in, out_ext, abrsbp, xrsp, outstp, tpool,
                  rs_c, c, rep):
    """Post-ReduceScatter: + bias_mean, transpose back, + residual, store."""
    rsb = abrsbp.tile([128, FSLICE // 128, NB], f32,
                      tag="abrsb", name=f"rsb{rep}_{c}")
    nc.gpsimd.dma_start(
        out=rsb, in_=rs_c.rearrange("(o p) n -> p o n", p=128))
    for j in range(FSLICE // 128):
        nc.scalar.activation(
            rsb[:, j, :], rsb[:, j, :], AF.Identity,
            bias=bm_sb[:, j:j + 1])
    for bp in range(NB // 256):
        r0 = c * NB + bp * 256
        xrs = xrsp.tile([128, 2, FSLICE], f32, tag="xrs")
        nc.sync.dma_start(
            out=xrs,
            in_=xres_in[r0:r0 + 256, :].rearrange("(o p) f -> p o f", p=128))
        obl = outstp.tile([128, 2, FSLICE // 128, 128], f32, tag="obl")
        for o in range(2):
            bsub = bp * 2 + o
            for j in range(FSLICE // 128):
                pt = tpool.tile([128, 128], f32, tag="tp")
                nc.tensor.transpose(
                    pt, rsb[:, j, bsub * 128:(bsub + 1) * 128], ident)
                nc.vector.tensor_tensor(
                    out=obl[:, o, j, :], in0=pt,
                    in1=xrs[:, o, j * 128:(j + 1) * 128], op=OP.add)
        nc.gpsimd.dma_start(
            out=out_ext[r0:r0 + 256, :].rearrange("(o p) f -> p o f", p=128),
            in_=obl.rearrange("p o a b -> p o (a b)"))


# revision 27
# speedup vs baseline: 24.8298x; 1.0319x over previous
"""GATv2 self-attention kernel for 8 Trainium2 NeuronCores.

Sharding: one attention head per core (8 heads / 8 cores). Each core computes
its head's attn-weighted projection as a partial sum over heads, the cores
ReduceScatter the partials over the feature axis, and each core finishes its
256-column feature slice (bias-mean + residual) and returns it; the host
concatenates the 8 slices.

Math per head h (reference):
  X = inputs.reshape(B*S, F); x0 = rows of X with s == 0
  Wh = leaky_relu(X @ W2h + broadcast_s(x0 @ W1h))      [B*S, F]
  e  = Wh @ att_w[h]; attn = softmax_s(e)
  out = sum_h (attn * Wh)/H + mean_h(bias) + X

All matmuls run in float32r (TF32-like, 11-bit mantissa) on the PE array.
X is transposed on-chip via PE transpose-mode matmuls. The broadcast x0@W1
term is accumulated into the same PSUM group as the X@W2 tiles through a
0/1 selector matmul, so no elementwise broadcast-add pass is needed.
"""

import sys
import os
import numpy as np

sys.path.insert(0, "/opt/trn_rl_repo")

B, S, F, H = 256, 8, 2048, 8
BS = B * S            # 2048
NB = 1024             # bs-chunk size (2 chunks)
NCHUNK = BS // NB     # 2
FSLICE = F // H       # 256 output feature columns per core
ALPHA = 0.3
USE_PRELU = True

_cache = {}


def _build(reps=1):
    import concourse.bacc as bacc
    import concourse.mybir as mybir
    import concourse.tile as tile
    import concourse.bass as bass
    from concourse.masks import make_identity

    f32 = mybir.dt.float32
    f32r = mybir.dt.float32r
    AF = mybir.ActivationFunctionType
    OP = mybir.AluOpType

    nc = bacc.Bacc(num_devices=H)

    w1_in = nc.declare_dram_parameter(
        "w1t", [F // 256, 128, 2, F // 128, 128], f32, isOutput=False)
    w2_in = nc.declare_dram_parameter(
        "w2t", [F // 256, 128, 2, F // 128, 128], f32, isOutput=False)
    x_in = nc.declare_dram_parameter("x", [BS, F], f32, isOutput=False)
    attw_in = nc.declare_dram_parameter("attw", [F], f32, isOutput=False)
    sel_in = nc.declare_dram_parameter("sel", [128, NB], f32, isOutput=False)
    xres_in = nc.declare_dram_parameter("xres", [BS, FSLICE], f32, isOutput=False)
    bm_in = nc.declare_dram_parameter("bm", [FSLICE], f32, isOutput=False)
    out_ext = nc.declare_dram_parameter("out", [BS, FSLICE], f32, isOutput=True)

    from contextlib import ExitStack
    with tile.TileContext(nc) as tc:
        with ExitStack() as ctx:
            consts = ctx.enter_context(tc.tile_pool(name="consts", bufs=1))
            xtp = ctx.enter_context(tc.tile_pool(name="xtp", bufs=1))
            whp = ctx.enter_context(tc.tile_pool(name="whp", bufs=1))
            wblkp = ctx.enter_context(tc.tile_pool(name="wblk", bufs=1))
            wrp = ctx.enter_context(tc.tile_pool(name="wrp", bufs=2))
            x0cp = ctx.enter_context(tc.tile_pool(name="x0c", bufs=1))
            xnatp = ctx.enter_context(tc.tile_pool(name="xnat", bufs=2))
            t03p = ctx.enter_context(tc.tile_pool(name="t03", bufs=2))
            esmp = ctx.enter_context(tc.tile_pool(name="esm", bufs=1))
            abrsbp = ctx.enter_context(tc.tile_pool(name="abrsb", bufs=1))
            xrsp = ctx.enter_context(tc.tile_pool(name="xrs", bufs=1))
            outstp = ctx.enter_context(tc.tile_pool(name="outst", bufs=1))
            ypool = ctx.enter_context(tc.tile_pool(name="ypool", bufs=4, space="PSUM"))
            epool = ctx.enter_context(tc.tile_pool(name="epool", bufs=2, space="PSUM"))
            tpool = ctx.enter_context(tc.tile_pool(name="tpool", bufs=2, space="PSUM"))
            dpool = ctx.enter_context(tc.tile_pool(name="dram", bufs=2, space="DRAM"))

            # ---------------- constants ----------------
            ident = consts.tile([128, 128], f32)
            make_identity(nc, ident)

            attw_f = consts.tile([128, F // 128], f32)
            nc.sync.dma_start(out=attw_f, in_=attw_in.rearrange("(o p) -> p o", p=128))
            attw_r = consts.tile([128, F // 128], f32r)
            nc.vector.tensor_copy(attw_r, attw_f)

            sel_r = consts.tile([128, NB], f32r)
            sel_stg = xnatp.tile([128, F], f32, tag="xnat")
            nc.sync.dma_start(out=sel_stg[:, :NB], in_=sel_in[:, :])
            nc.vector.tensor_copy(sel_r, sel_stg[:, :NB])

            al_sb = consts.tile([128, 1], f32)
            nc.vector.memset(al_sb, ALPHA)

            bm_sb = consts.tile([128, FSLICE // 128], f32)
            nc.sync.dma_start(out=bm_sb, in_=bm_in.rearrange("(o p) -> p o", p=128))

            # view of x grouped by (b, s)
            x_bsf = x_in.rearrange("(b s) f -> b s f", s=S)

            for _rep in range(reps):
                _run_body(nc, tc, mybir, bass, f32, f32r, AF, OP,
                          make_identity, ident, attw_r, sel_r, bm_sb, al_sb, x_bsf,
                          w1_in, w2_in, x_in, xres_in, out_ext,
                          consts, xtp, whp, wblkp, wrp, x0cp, xnatp, t03p,
                          esmp, abrsbp, xrsp, outstp,
                          ypool, epool, tpool, dpool, _rep)

    nc.compile()
    return nc


def _run_body(nc, tc, mybir, bass, f32, f32r, AF, OP,
              make_identity, ident, attw_r, sel_r, bm_sb, al_sb, x_bsf,
              w1_in, w2_in, x_in, xres_in, out_ext,
              consts, xtp, whp, wblkp, wrp, x0cp, xnatp, t03p,
              esmp, abrsbp, xrsp, outstp,
              ypool, epool, tpool, dpool, rep):
    NFB = F // 128   # 16 fo/fi blocks
    # ---------------- prologue: X0 = x0 @ W1 ----------------
    # x0t: [128 fi, 16 fi_outer, 256 b] carved out of the Wh buffer slot
    x0t_full = whp.tile([128, NFB, NB], f32r, tag="wh")
    x0t = x0t_full[:, :, :B]
    for bt in range(B // 128):
        x0nat = xnatp.tile([128, F], f32, tag="xnat")
        nc.sync.dma_start(
            out=x0nat, in_=x_bsf[bt * 128:(bt + 1) * 128, 0, :])
        for fi in range(NFB):
            pt = tpool.tile([128, 128], f32, tag="tp")
            nc.tensor.transpose(
                pt, x0nat[:, fi * 128:(fi + 1) * 128], ident)
            nc.any.tensor_copy(
                out=x0t[:, fi, bt * 128:(bt + 1) * 128], in_=pt)

    # X0 in transposed layout first: X0_T [128 fo, 16 fo_outer, 256 b]
    x0T = x0t_full[:, :, B:2 * B]
    for pair in range(NFB // 2):
        wblk = wblkp.tile([128, 2, NFB, 128], f32, tag="wblk")
        nc.sync.dma_start(out=wblk, in_=w1_in[pair])
        for half in range(2):
            fb = pair * 2 + half
            wr = wrp.tile([128, NFB, 128], f32r, tag="wr")
            nc.vector.tensor_copy(wr, wblk[:, half])
            ps_full = ypool.tile([128, 512], f32, tag="yp")
            ps = ps_full[:, :B]
            for fi in range(NFB):
                nc.tensor.matmul(
                    ps, wr[:, fi, :], x0t[:, fi, :],
                    start=(fi == 0), stop=(fi == NFB - 1))
            nc.any.tensor_copy(out=x0T[:, fb, :], in_=ps)

    # transpose X0_T back to natural layout, stage to DRAM (fp32)
    x0_dram = dpool.tile([B // 128, 128, F], f32, tag="x0dram")
    for bt in range(B // 128):
        x0stg = xnatp.tile([128, F], f32, tag="xnat")
        for fb in range(NFB):
            pt = tpool.tile([128, 128], f32, tag="tp")
            nc.tensor.transpose(
                pt, x0T[:, fb, bt * 128:(bt + 1) * 128].bitcast(f32), ident)
            nc.any.tensor_copy(
                out=x0stg[:, fb * 128:(fb + 1) * 128], in_=pt)
        nc.sync.dma_start(out=x0_dram[bt], in_=x0stg)

    # ---------------- main loop over bs-chunks ----------------
    rs_list = []
    for c in range(NCHUNK):
        # -- build x_t chunk [128 fi, 16 fi_outer, NB bs] (f32r) --
        x_t = xtp.tile([128, NFB, NB], f32r, tag="xt")
        for bsub in range(NB // 128):
            r0 = c * NB + bsub * 128
            xnat = xnatp.tile([128, F], f32, tag="xnat")
            eng = nc.sync if bsub % 2 == 0 else nc.gpsimd
            eng.dma_start(out=xnat, in_=x_in[r0:r0 + 128, :])
            for fi in range(NFB):
                pt = tpool.tile([128, 128], f32, tag="tp")
                nc.tensor.transpose(
                    pt, xnat[:, fi * 128:(fi + 1) * 128], ident)
                nc.any.tensor_copy(
                    out=x_t[:, fi, bsub * 128:(bsub + 1) * 128], in_=pt)

        # X0 rows for this chunk: load + round to f32r
        x0ld = xnatp.tile([128, F], f32, tag="xnat")
        nc.sync.dma_start(out=x0ld, in_=x0_dram[c])
        x0c_r = x0cp.tile([128, F], f32r, tag="x0c", name=f"x0c{rep}_{c}")
        nc.vector.tensor_copy(x0c_r, x0ld)

        # -- Wh chunk [128 fo, 16 fo_outer, NB bs] --
        wh = whp.tile([128, NFB, NB], f32r, tag="wh")
        e_ps = []
        for _b2 in range(NB // 512):
            e_ps_t = epool.tile([1, 512], f32, tag="ep",
                                name=f"eps{rep}_{c}_{_b2}")
            e_ps.append(e_ps_t)

        pending_e = []  # lag e-matvecs one fo-block so PE never waits on ACT
        for pair in range(NFB // 2):
            wblk = wblkp.tile([128, 2, NFB, 128], f32, tag="wblk")
            nc.sync.dma_start(out=wblk, in_=w2_in[pair])
            for half in range(2):
                fo = pair * 2 + half
                wr = wrp.tile([128, NFB, 128], f32r, tag="wr")
                nc.vector.tensor_copy(wr, wblk[:, half])
                for b2 in range(NB // 512):
                    ps = ypool.tile([128, 512], f32, tag="yp")
                    for fi in range(NFB):
                        nc.tensor.matmul(
                            ps, wr[:, fi, :],
                            x_t[:, fi, b2 * 512:(b2 + 1) * 512],
                            start=(fi == 0), stop=False)
                    # + broadcast_s(X0): selector matmul closes the group
                    nc.tensor.matmul(
                        ps, x0c_r[:, fo * 128:(fo + 1) * 128],
                        sel_r[:, b2 * 512:(b2 + 1) * 512],
                        start=False, stop=True)
                    # leaky_relu on ACT (exact Prelu), writes f32r Wh
                    whs = wh[:, fo, b2 * 512:(b2 + 1) * 512]
                    if USE_PRELU:
                        nc.scalar.activation(whs, ps, AF.Prelu,
                                             alpha=al_sb[:, :])
                    else:
                        t03 = t03p.tile([128, 512], f32, tag="t03")
                        nc.scalar.activation(t03, ps, AF.Copy, scale=ALPHA)
                        nc.vector.tensor_tensor(
                            out=whs, in0=ps, in1=t03, op=OP.max)
                    pending_e.append((fo, b2, whs))
                # emit lagged e-matvecs (previous fo block)
                while len(pending_e) > NB // 512:
                    efo, eb2, ewhs = pending_e.pop(0)
                    nc.tensor.matmul(
                        e_ps[eb2], attw_r[:, efo:efo + 1], ewhs,
                        start=(efo == 0), stop=(efo == NFB - 1))
        for efo, eb2, ewhs in pending_e:
            nc.tensor.matmul(
                e_ps[eb2], attw_r[:, efo:efo + 1], ewhs,
                start=(efo == 0), stop=(efo == NFB - 1))

        # -- softmax over s (groups of 8 along bs), scaled by 1/H --
        ab_full = abrsbp.tile([128, FSLICE // 128, NB], f32,
                              tag="abrsb", name=f"abf{rep}_{c}")
        ab_sb = ab_full[:, 0, :]
        NG = 512 // S
        for b2 in range(NB // 512):
            e_sb = esmp.tile([1, 512], f32, tag="esb",
                             name=f"esb{rep}_{c}_{b2}")
            nc.scalar.activation(e_sb, e_ps[b2], AF.Copy)
            work = esmp.tile([1, 512], f32, tag="work",
                             name=f"work{rep}_{c}_{b2}")
            e3 = e_sb.rearrange("p (b s) -> p b s", s=S)
            w3 = work.rearrange("p (b s) -> p b s", s=S)
            mx = esmp.tile([1, NG], f32, tag="mx", name=f"mx{rep}_{c}_{b2}")
            nc.vector.reduce_max(out=mx, in_=e3, axis=mybir.AxisListType.X)
            nc.vector.tensor_tensor(
                out=w3, in0=e3, in1=mx[:, :, None].to_broadcast((1, NG, S)),
                op=OP.subtract)
            nc.scalar.activation(e_sb, work, AF.Exp)
            sm = esmp.tile([1, NG], f32, tag="sm", name=f"sm{rep}_{c}_{b2}")
            nc.vector.reduce_sum(out=sm, in_=e3, axis=mybir.AxisListType.X)
            rc = esmp.tile([1, NG], f32, tag="rc", name=f"rc{rep}_{c}_{b2}")
            nc.vector.reciprocal(rc, sm)
            nc.vector.tensor_scalar_mul(rc, rc, 1.0 / H)
            attn_sb = work
            nc.vector.tensor_tensor(
                out=w3, in0=e3, in1=rc[:, :, None].to_broadcast((1, NG, S)),
                op=OP.mult)
            attn_dram = dpool.tile([1, 512], f32, tag="attn_dram")
            nc.gpsimd.dma_start(out=attn_dram[:, :], in_=attn_sb)
            attn_bc = bass.AP(
                tensor=attn_dram.tensor,
                offset=attn_dram.offset,
                ap=[[0, 128]] + [list(p) for p in attn_dram[:, :].ap[1:]],
            )
            nc.gpsimd.dma_start(
                out=ab_sb[:, b2 * 512:(b2 + 1) * 512], in_=attn_bc)

        # -- partial = attn/H * Wh, in place, then 2 bulk DMAs --
        partial_c = dpool.tile([F, NB], f32, tag="partial")
        pview = partial_c.rearrange("(o p) n -> p o n", p=128)
        for fo in range(NFB):
            whs = wh[:, fo, :]
            nc.vector.tensor_tensor(
                out=whs, in0=whs.bitcast(f32), in1=ab_sb, op=OP.mult)
        for hh in range(2):
            nc.gpsimd.dma_start(
                out=pview[:, hh * 8:(hh + 1) * 8, :],
                in_=wh[:, hh * 8:(hh + 1) * 8, :].bitcast(f32))

        # -- reduce over heads: ReduceScatter along f-axis --
        rs_c = dpool.tile([FSLICE, NB], f32, tag="rs", name=f"rs{rep}_{c}")
        nc.gpsimd.collective_compute(
            "ReduceScatter", OP.add,
            replica_groups=[list(range(H))],
            ins=[partial_c[:, :]], outs=[rs_c[:, :]])
        rs_list.append(rs_c)

    # finish phase after all chunks (overlaps trailing collectives)
    for c in range(NCHUNK):
        _finish_chunk(nc, tc, mybir, bass, f32, f32r, AF, OP, ident, bm_sb,
                      xres_in, out_ext, abrsbp, xrsp, outstp, tpool,
                      rs_list[c], c, rep)


def _get_nc():
    if "nc" not in _cache:
        _cache["nc"] = _build()
    return _cache["nc"]


def _make_in_maps(inputs_dict):
    x = np.ascontiguousarray(
        np.asarray(inputs_dict["inputs"], dtype=np.float32).reshape(BS, F))
    W = np.asarray(inputs_dict["W"], dtype=np.float32)
    att_w = np.asarray(inputs_dict["att_w"], dtype=np.float32)
    bias = np.asarray(inputs_dict["bias"], dtype=np.float32)

    sel = np.repeat(np.eye(128, dtype=np.float32), S, axis=1)  # [128, 1024]
    bm_full = bias.mean(axis=0)  # [F]

    def tile_w(w):
        # [F, F] -> [pair, kp, b, ko, n]; fi = ko*128+kp, fo = pair*256+b*128+n
        t = w.reshape(F // 128, 128, F // 256, 2, 128)  # [ko, kp, pair, b, n]
        return np.ascontiguousarray(t.transpose(2, 1, 3, 0, 4))

    in_maps = []
    for i in range(H):
        in_maps.append({
            "w1t": tile_w(W[i, :F, :]),
            "w2t": tile_w(W[i, F:, :]),
            "x": x,
            "attw": np.ascontiguousarray(att_w[i]),
            "sel": sel,
            "xres": np.ascontiguousarray(x[:, FSLICE * i:FSLICE * (i + 1)]),
            "bm": np.ascontiguousarray(bm_full[FSLICE * i:FSLICE * (i + 1)]),
        })
    return in_maps


def kernel(inputs, W, att_w, bias):
    from concourse.bass_utils import run_bass_kernel_spmd

    nc = _get_nc()
    in_maps = _make_in_maps(
        {"inputs": inputs, "W": W, "att_w": att_w, "bias": bias})
    res = run_bass_kernel_spmd(nc, in_maps, list(range(H)))
    _cache["last_result"] = res

    out = np.concatenate([res.results[i]["out"] for i in range(H)], axis=1)
    return out.reshape(B, S, F)


def _finish_chunk(nc, tc, mybir, bass, f32, f32r, AF, OP, ident, bm_sb,
                  xres_in, out_ext, abrsbp, xrsp, outstp, tpool,
                  rs_c, c, rep):
    """Post-ReduceScatter: + bias_mean, transpose back, + residual, store."""
    rsb = abrsbp.tile([128, FSLICE // 128, NB], f32,
                      tag="abrsb", name=f"rsb{rep}_{c}")
    nc.gpsimd.dma_start(
        out=rsb, in_=rs_c.rearrange("(o p) n -> p o n", p=128))
    for j in range(FSLICE // 128):
        nc.scalar.activation(
            rsb[:, j, :], rsb[:, j, :], AF.Identity,
            bias=bm_sb[:, j:j + 1])
    for bp in range(NB // 256):
        r0 = c * NB + bp * 256
        xrs = xrsp.tile([128, 2, FSLICE], f32, tag="xrs")
        nc.sync.dma_start(
            out=xrs,
            in_=xres_in[r0:r0 + 256, :].rearrange("(o p) f -> p o f", p=128))
        obl = outstp.tile([128, 2, FSLICE // 128, 128], f32, tag="obl")
        for o in range(2):
            bsub = bp * 2 + o
            for j in range(FSLICE // 128):
                pt = tpool.tile([128, 128], f32, tag="tp")
                nc.tensor.transpose(
                    pt, rsb[:, j, bsub * 128:(bsub + 1) * 128], ident)
                nc.vector.tensor_tensor(
                    out=obl[:, o, j, :], in0=pt,
                    in1=xrs[:, o, j * 128:(j + 1) * 128], op=OP.add)
        nc.gpsimd.dma_start(
            out=out_ext[r0:r0 + 256, :].rearrange("(o p) f -> p o f", p=128),
            in_=obl.rearrange("p o a b -> p o (a b)"))
